# revision 28
# baseline (speedup 1.0000x reference)
"""Multi-head causal attention (bs=4, L=2048, d_model=512, 8 heads x 64) on 8
Trainium2 NeuronCores.

Sharding: core c = (batch b = c//2, head-group hg = c%2); each core computes 4
heads of one batch over the full sequence. Host pre-transposes activations and
weight slices so every device matmul has its contraction dim on partitions;
device returns the transposed partial output projection; host sums the two
head-group partials per batch, transposes back and adds the (folded) biases.

v2 layout: 256-wide query blocks (8 of them) for tighter causal coverage.
Per block and head-pair, scores for both heads of a pair land in one 2-bank
PSUM tile [128, 2, 2, 256] so a single ACT exp op covers 2 heads x 2 key
tiles.  The causal mask is a constant 0/1 f16 tile multiplied in on DVE
(diagonal group only).  Softmax denominators ride the z matmul as a ones
column (row 64); normalization: den row -> DMA to partition 0 -> fast
reciprocal -> gpsimd partition_broadcast -> DVE mul straight out of PSUM.
Projection work (QKV, output) is emitted lazily between score groups so the
PE always has dependency-free matmuls to chew on while ACT runs exp.
"""

import numpy as np

import concourse.bacc as bacc
import concourse.mybir as mybir
import concourse.tile as tile
from concourse.bass_utils import run_bass_kernel_spmd

F32 = mybir.dt.float32
F32R = mybir.dt.float32r
F16 = mybir.dt.float16
AF = mybir.ActivationFunctionType

L = 2048          # sequence length
D = 512           # model dim
HD = 256          # head-group output dim (4 heads x 64)
DK = 64           # head dim
P = 128
IB = 256          # query block width
NB = L // IB      # 8 query blocks
NKT = D // P      # 4 contraction tiles over model dim
NJT = L // P      # 16 key tiles
SCALE = 1.0 / 8.0  # 1/sqrt(DK)


def _build():
    nc = bacc.Bacc("TRN2", target_bir_lowering=False, debug=False,
                   enable_asserts=False)

    xT = nc.dram_tensor("xT", [D, L], F16, kind="ExternalInput")
    wq = nc.dram_tensor("wq", [D, HD], F16, kind="ExternalInput")
    wk = nc.dram_tensor("wk", [D, HD], F16, kind="ExternalInput")
    wv = nc.dram_tensor("wv", [D, HD], F16, kind="ExternalInput")
    wo = nc.dram_tensor("wo", [HD, D], F16, kind="ExternalInput")
    bq = nc.dram_tensor("bq", [HD], F32, kind="ExternalInput")
    bk = nc.dram_tensor("bk", [HD], F32, kind="ExternalInput")
    outT = nc.dram_tensor("outT", [D, L], F32, kind="ExternalOutput")

    with tile.TileContext(nc) as tc:
        with (
            tc.tile_pool(name="w", bufs=1) as pool_w,
            tc.tile_pool(name="x", bufs=1) as pool_x,
            tc.tile_pool(name="qk", bufs=1) as pool_qk,
            tc.tile_pool(name="v", bufs=1) as pool_v,
            tc.tile_pool(name="at", bufs=3) as pool_at,
            tc.tile_pool(name="nm", bufs=2) as pool_nm,
            tc.tile_pool(name="zc", bufs=4) as pool_zc,
            tc.tile_pool(name="o", bufs=2) as pool_o,
            tc.tile_pool(name="ps", bufs=1, space="PSUM") as pool_ps,
            tc.tile_pool(name="pz", bufs=4, space="PSUM") as pool_pz,
            tc.tile_pool(name="pp", bufs=2, space="PSUM") as pool_pp,
        ):
            # ---- static tiles ----
            wq_sb = pool_w.tile([P, NKT, HD], F16, tag="wq")
            wk_sb = pool_w.tile([P, NKT, HD], F16, tag="wk")
            wv_sb = pool_w.tile([P, NKT, HD], F16, tag="wv")
            wo_sb = pool_w.tile([P, HD // P, D], F16, tag="wo")
            bq_sb = pool_w.tile([P, HD // P], F32, tag="bq")
            bk_sb = pool_w.tile([P, HD // P], F32, tag="bk")
            # interleave weight and x-chunk arrivals so the first projection
            # matmuls can start as soon as wq + one x column-chunk land
            xts = pool_x.tile([P, NKT, L], F16, tag="x")

            def dma_x(b):
                for kt in range(NKT):
                    nc.sync.dma_start(
                        xts[:, kt, b * IB:(b + 1) * IB],
                        xT.ap()[kt * P:(kt + 1) * P, b * IB:(b + 1) * IB])

            nc.sync.dma_start(wq_sb[:], wq.ap().rearrange("(t p) n -> p t n", p=P))
            nc.sync.dma_start(bq_sb[:], bq.ap().rearrange("(t p) -> p t", p=P))
            dma_x(0)
            dma_x(1)
            nc.sync.dma_start(wk_sb[:], wk.ap().rearrange("(t p) n -> p t n", p=P))
            nc.sync.dma_start(bk_sb[:], bk.ap().rearrange("(t p) -> p t", p=P))
            dma_x(2)
            nc.sync.dma_start(wv_sb[:], wv.ap().rearrange("(t p) n -> p t n", p=P))
            dma_x(3)
            nc.sync.dma_start(wo_sb[:], wo.ap().rearrange("(t p) n -> p t n", p=P))
            for b in range(4, NB):
                dma_x(b)

            # causal mask for the diagonal k-tile pair: keep iff i >= 128*t + p
            maskT = pool_w.tile([P, 2, IB], F16, tag="maskT")
            nc.gpsimd.memset(maskT[:], 1.0)
            nc.gpsimd.affine_select(
                maskT[:], maskT[:],
                pattern=[[-P, 2], [1, IB]],
                compare_op=mybir.AluOpType.is_ge,
                fill=0.0, base=0, channel_multiplier=-1,
            )

            # q/k per d-tile: [128, L] f16 (rows 0:64 head-even, 64:128 head-odd)
            qk_sb = {}
            for nm in ("q", "k"):
                for dt in range(2):
                    qk_sb[(nm, dt)] = pool_qk.tile([P, L], F16, tag=f"{nm}{dt}",
                                                   name=f"{nm}{dt}")
            # v: [128, jt, head, 65] with ones in col 64
            v_sb = pool_v.tile([P, NJT, 4, DK + 1], F16, tag="v")
            nc.gpsimd.memset(v_sb[:, :, :, DK:DK + 1], 1.0)
            # f32 ones row for the PE denominator-broadcast matmul
            ones_r = pool_w.tile([1, DK], F32, tag="ones_r")
            nc.gpsimd.memset(ones_r[:], 1.0)

            # ---- projection emitters (called lazily) ----
            def emit_qk_block(nm, dt, b, nb=1):
                """project q or k for d-tile dt, query blocks [b, b+nb)."""
                w_sb, b_sb = (wq_sb, bq_sb) if nm == "q" else (wk_sb, bk_sb)
                w_ = IB * nb
                pp = pool_pp.tile([P, 2, IB], F32, tag="pp", name="pp")
                ppv = pp[:].rearrange("p a f -> p (a f)")[:, 0:w_]
                for kt in range(NKT):
                    nc.tensor.matmul(
                        ppv,
                        lhsT=w_sb[:, kt, dt * P:(dt + 1) * P],
                        rhs=xts[:, kt, b * IB:b * IB + w_],
                        start=(kt == 0), stop=(kt == NKT - 1),
                    )
                nc.vector.tensor_scalar_add(
                    qk_sb[(nm, dt)][:, b * IB:b * IB + w_], ppv,
                    b_sb[:, dt:dt + 1])

            def emit_v(jt):
                pp = pool_pp.tile([P, 2, IB], F32, tag="pp", name="pp")
                ppv = pp[:, 0, :]
                for kt in range(NKT):
                    nc.tensor.matmul(
                        ppv,
                        lhsT=xts[:, kt, jt * P:(jt + 1) * P],
                        rhs=wv_sb[:, kt, :],
                        start=(kt == 0), stop=(kt == NKT - 1),
                    )
                nc.vector.tensor_copy(
                    v_sb[:, jt, :, 0:DK],
                    ppv.rearrange("p (h e) -> p h e", h=4))

            # ---- attention ----
            zcs = {}

            def attn_pair(b, p, fillers):
                """scores+exp+mask+z for head pair p (heads 2p, 2p+1) of
                block b.  `fillers` is a list of (deadline, fn) lazy emitters
                (projection chunks) interleaved between score groups to keep
                the PE fed while ACT catches up.  Anything due by (b, p) is
                emitted up front -- a PE consumer emitted before its PE
                producer would deadlock the in-order engine stream."""
                due = [f for dl, f in fillers if dl <= (b, p)]
                fillers[:] = [(dl, f) for dl, f in fillers if not dl <= (b, p)]
                for f in due:
                    f()
                nj = 2 * (b + 1)
                qt = qk_sb[("q", p)]
                kt_t = qk_sb[("k", p)]
                # one PSUM bank per head: an open accumulation group must own
                # its bank (start=True clears has_written bank-wide)
                pzc = [pool_pz.tile([P, IB], F32, tag="pz", name=f"pz{c}")
                       for c in range(2)]
                zcs[(b, p)] = (pzc, pool_zc.tile([P, IB], F16, tag="zc",
                                                 name="zc"))
                prev = None

                def emit_z(g, jbs, at):
                    for s, jb in enumerate(jbs):
                        for c in range(2):
                            nc.tensor.matmul(
                                pzc[c][0:DK + 1, :],
                                lhsT=v_sb[:, jb, 2 * p + c, :],
                                rhs=at[:, c, s, :],
                                start=(jb == 0), stop=(jb == nj - 1),
                            )

                ngrp = (nj + 1) // 2
                for g in range(ngrp):
                    jbs = [2 * g + s for s in range(2) if 2 * g + s < nj]
                    ps = pool_ps.tile([P, 2, 2, IB], F32, tag="ps", name="ps")
                    at = pool_at.tile([P, 2, 2, IB], F16, tag="at", name="at")
                    for s, jb in enumerate(jbs):
                        for c in range(2):
                            drow = DK * c
                            nc.tensor.matmul(
                                ps[:, c, s, :],
                                lhsT=kt_t[drow:drow + DK,
                                          jb * P:(jb + 1) * P],
                                rhs=qt[drow:drow + DK,
                                       b * IB:(b + 1) * IB],
                                start=True, stop=True,
                            )
                    if fillers:
                        fillers.pop(0)[1]()
                    if prev is not None:
                        emit_z(*prev)
                    nc.scalar.activation(
                        at[:].rearrange("p a b f -> p (a b f)"),
                        ps[:].rearrange("p a b f -> p (a b f)"),
                        AF.Exp, scale=SCALE)
                    if g == ngrp - 1:
                        # diagonal pair: zero at[j, i] where i < 128*s + j
                        for c in range(2):
                            nc.vector.tensor_mul(at[:, c, :, :],
                                                 at[:, c, :, :], maskT[:])
                    prev = (g, jbs, at)
                emit_z(*prev)

            def norm_pair(b, p):
                """divide z by the softmax denominator (row 64 of pz).
                z+den move to SBUF at once (frees the pz banks early); the
                den row DMA-hops to partition 0 for the fast reciprocal; the
                reciprocal row is broadcast to 64 partitions by a tiny f32r
                PE matmul with a ones column (PSUM out), so the final DVE
                muls read one SBUF + one PSUM operand, all at base 0."""
                pzc, zc = zcs[(b, p)]
                zsb = pool_nm.tile([P, 2, IB], F32, tag="zsb", name="zsb")
                dent = pool_nm.tile([1, 2, IB], F32, tag="dent", name="dent")
                rden = pool_nm.tile([1, 2, IB], F32, tag="rden", name="rden")
                zn = pool_nm.tile([DK, IB], F16, tag="zn", name="zn")
                for c in range(2):
                    nc.vector.tensor_copy(zsb[0:DK + 1, c, :],
                                          pzc[c][0:DK + 1, :])
                nc.gpsimd.dma_start(dent[:], zsb[DK:DK + 1, :, :])
                nc.vector.reciprocal_approx_fast(rden[:], dent[:])
                bcts = []
                for c in range(2):
                    bct = pool_pz.tile([P, IB], F32, tag="pz", name="bct")
                    nc.tensor.matmul(
                        bct[0:DK, :],
                        lhsT=ones_r[:],
                        rhs=rden[0:1, c, :],
                        start=True, stop=True,
                    )
                    bcts.append(bct)
                # head-even -> zc rows 0:64; head-odd via SBUF DMA hop
                nc.vector.tensor_mul(zc[0:DK, :], zsb[0:DK, 0, :],
                                     bcts[0][0:DK, :])
                nc.vector.tensor_mul(zn[:], zsb[0:DK, 1, :], bcts[1][0:DK, :])
                nc.gpsimd.dma_start(zc[DK:P, :], zn[:])

            def outproj_mt(b, mt):
                po = pool_pp.tile([P, IB], F32, tag="pp", name="pp")
                for p in range(2):
                    nc.tensor.matmul(
                        po[:],
                        lhsT=wo_sb[:, p, mt * P:(mt + 1) * P],
                        rhs=zcs[(b, p)][1][:],
                        start=(p == 0), stop=(p == 1),
                    )
                osb = pool_o.tile([P, IB], F32, tag="o", name="o")
                nc.vector.tensor_copy(osb[:], po[:])
                nc.sync.dma_start(
                    outT.ap()[mt * P:(mt + 1) * P, b * IB:(b + 1) * IB],
                    osb[:])

            # ---- emission schedule ----
            # prologue: q/k for blocks 0-1 (512-wide chunks), v for block 0
            for dt in range(2):
                emit_qk_block("q", dt, 0, nb=2)
                emit_qk_block("k", dt, 0, nb=2)
            emit_v(0)
            emit_v(1)

            # global lazy-work queue of (deadline, fn): one item pops per
            # score group so the PE always has dependency-free matmuls while
            # ACT runs exp; anything still pending at its deadline is
            # force-emitted by attn_pair
            END = (NB, 2)
            fillq = []
            for b in range(NB):
                if b + 1 < NB:
                    fillq.append(((b + 1, 0), lambda b=b: emit_v(2 * b + 2)))
                    fillq.append(((b + 1, 0), lambda b=b: emit_v(2 * b + 3)))
                if b % 2 == 0 and b + 2 < NB:
                    # 512-wide q/k chunks for blocks b+2, b+3
                    for dt in range(2):
                        fillq.append(((b + 2, dt),
                                      lambda dt=dt, b=b:
                                      emit_qk_block("q", dt, b + 2, nb=2)))
                        fillq.append(((b + 2, dt),
                                      lambda dt=dt, b=b:
                                      emit_qk_block("k", dt, b + 2, nb=2)))
                if b >= 1:
                    for mt in range(D // P):
                        fillq.append((END,
                                      lambda mt=mt, b=b: outproj_mt(b - 1, mt)))
                attn_pair(b, 0, fillq)
                norm_pair(b, 0)
                attn_pair(b, 1, fillq)
                norm_pair(b, 1)
            for _, f in fillq:
                f()
            for mt in range(D // P):
                outproj_mt(NB - 1, mt)

    nc.compile()
    return nc


_NC = None


def _get_nc():
    global _NC
    if _NC is None:
        _NC = _build()
    return _NC


def _in_maps(x, w_q, b_q, w_k, b_k, w_v, b_v, w_o, b_o):
    maps = []
    for b in range(4):
        xTb = np.ascontiguousarray(x[b].T.astype(np.float16))
        for hg in range(2):
            sl = slice(hg * HD, (hg + 1) * HD)
            maps.append({
                "xT": xTb,
                "wq": np.ascontiguousarray(w_q[sl].T.astype(np.float16)),
                "wk": np.ascontiguousarray(w_k[sl].T.astype(np.float16)),
                "wv": np.ascontiguousarray(w_v[sl].T.astype(np.float16)),
                "wo": np.ascontiguousarray(w_o[:, sl].T.astype(np.float16)),
                "bq": np.ascontiguousarray(b_q[sl].astype(np.float32)),
                "bk": np.ascontiguousarray(b_k[sl].astype(np.float32)),
            })
    return maps


def _combine(results, w_o, b_v, b_o):
    corr = (b_o + w_o @ b_v).astype(np.float32)  # fold v/out biases
    out = np.empty((4, L, D), dtype=np.float32)
    for b in range(4):
        acc = results[2 * b]["outT"] + results[2 * b + 1]["outT"]
        out[b] = acc.T + corr
    return out


def kernel(x, w_q, b_q, w_k, b_k, w_v, b_v, w_o, b_o):
    nc = _get_nc()
    maps = _in_maps(x, w_q, b_q, w_k, b_k, w_v, b_v, w_o, b_o)
    res = run_bass_kernel_spmd(nc, maps, core_ids=list(range(8)))
    return _combine(res.results, w_o, b_v, b_o)


def bench(x, w_q, b_q, w_k, b_k, w_v, b_v, w_o, b_o):
    """Run with NTFF tracing; returns (output, exec_time_ns)."""
    nc = _get_nc()
    maps = _in_maps(x, w_q, b_q, w_k, b_k, w_v, b_v, w_o, b_o)
    res = run_bass_kernel_spmd(nc, maps, core_ids=list(range(8)), trace=True)
    return _combine(res.results, w_o, b_v, b_o), res.exec_time_ns


# revision 31
# speedup vs baseline: 1.2123x; 1.2123x over previous
"""Multi-head causal attention (bs=4, L=2048, d_model=512, 8 heads x 64) on 8
Trainium2 NeuronCores.

Sharding: core c = (batch b = c//2, head-group hg = c%2); each core computes 4
heads of one batch over the full sequence. Host pre-transposes activations and
weight slices so every device matmul has its contraction dim on partitions;
device returns the transposed partial output projection; host sums the two
head-group partials per batch, transposes back and adds the (folded) biases.

v2 layout: 256-wide query blocks (8 of them) for tighter causal coverage.
Per block and head-pair, scores for both heads of a pair land in one 2-bank
PSUM tile [128, 2, 2, 256] so a single ACT exp op covers 2 heads x 2 key
tiles.  The causal mask is a constant 0/1 f16 tile multiplied in on DVE
(diagonal group only).  Softmax denominators ride the z matmul as a ones
column (row 64); normalization: den row -> DMA to partition 0 -> fast
reciprocal -> gpsimd partition_broadcast -> DVE mul straight out of PSUM.
Projection work (QKV, output) is emitted lazily between score groups so the
PE always has dependency-free matmuls to chew on while ACT runs exp.
"""

import numpy as np

import concourse.bacc as bacc
import concourse.mybir as mybir
import concourse.tile as tile
from concourse.bass_utils import run_bass_kernel_spmd

F32 = mybir.dt.float32
F32R = mybir.dt.float32r
F16 = mybir.dt.float16
AF = mybir.ActivationFunctionType

L = 2048          # sequence length
D = 512           # model dim
HD = 256          # head-group output dim (4 heads x 64)
DK = 64           # head dim
P = 128
IB = 256          # query block width
NB = L // IB      # 8 query blocks
NKT = D // P      # 4 contraction tiles over model dim
NJT = L // P      # 16 key tiles
SCALE = 1.0 / 8.0  # 1/sqrt(DK)


def _build():
    nc = bacc.Bacc("TRN2", target_bir_lowering=False, debug=False,
                   enable_asserts=False)

    xT = nc.dram_tensor("xT", [D, L], F16, kind="ExternalInput")
    wq = nc.dram_tensor("wq", [D, HD], F16, kind="ExternalInput")
    wk = nc.dram_tensor("wk", [D, HD], F16, kind="ExternalInput")
    wv = nc.dram_tensor("wv", [D, HD], F16, kind="ExternalInput")
    wo = nc.dram_tensor("wo", [HD, D], F16, kind="ExternalInput")
    bq = nc.dram_tensor("bq", [HD], F32, kind="ExternalInput")
    bk = nc.dram_tensor("bk", [HD], F32, kind="ExternalInput")
    outT = nc.dram_tensor("outT", [D, L], F32, kind="ExternalOutput")

    with tile.TileContext(nc) as tc:
        with (
            tc.tile_pool(name="w", bufs=1) as pool_w,
            tc.tile_pool(name="x", bufs=1) as pool_x,
            tc.tile_pool(name="qk", bufs=1) as pool_qk,
            tc.tile_pool(name="v", bufs=1) as pool_v,
            tc.tile_pool(name="at", bufs=3) as pool_at,
            tc.tile_pool(name="nm", bufs=3) as pool_nm,
            tc.tile_pool(name="zc", bufs=4) as pool_zc,
            tc.tile_pool(name="o", bufs=2) as pool_o,
            tc.tile_pool(name="ps", bufs=1, space="PSUM") as pool_ps,
            tc.tile_pool(name="pz", bufs=4, space="PSUM") as pool_pz,
            tc.tile_pool(name="pp", bufs=2, space="PSUM") as pool_pp,
        ):
            # ---- static tiles ----
            wq_sb = pool_w.tile([P, NKT, HD], F16, tag="wq")
            wk_sb = pool_w.tile([P, NKT, HD], F16, tag="wk")
            wv_sb = pool_w.tile([P, NKT, HD], F16, tag="wv")
            wo_sb = pool_w.tile([P, HD // P, D], F16, tag="wo")
            bq_sb = pool_w.tile([P, HD // P], F32, tag="bq")
            bk_sb = pool_w.tile([P, HD // P], F32, tag="bk")
            # interleave weight and x-chunk arrivals so the first projection
            # matmuls can start as soon as wq + one x column-chunk land
            xts = pool_x.tile([P, NKT, L], F16, tag="x")

            def dma_x(b):
                for kt in range(NKT):
                    nc.sync.dma_start(
                        xts[:, kt, b * IB:(b + 1) * IB],
                        xT.ap()[kt * P:(kt + 1) * P, b * IB:(b + 1) * IB])

            nc.sync.dma_start(wq_sb[:], wq.ap().rearrange("(t p) n -> p t n", p=P))
            nc.sync.dma_start(bq_sb[:], bq.ap().rearrange("(t p) -> p t", p=P))
            dma_x(0)
            dma_x(1)
            nc.sync.dma_start(wk_sb[:], wk.ap().rearrange("(t p) n -> p t n", p=P))
            nc.sync.dma_start(bk_sb[:], bk.ap().rearrange("(t p) -> p t", p=P))
            dma_x(2)
            nc.sync.dma_start(wv_sb[:], wv.ap().rearrange("(t p) n -> p t n", p=P))
            dma_x(3)
            nc.sync.dma_start(wo_sb[:], wo.ap().rearrange("(t p) n -> p t n", p=P))
            for b in range(4, NB):
                dma_x(b)

            # causal mask for the diagonal k-tile pair: keep iff i >= 128*t + p
            maskT = pool_w.tile([P, 2, IB], F16, tag="maskT")
            nc.gpsimd.memset(maskT[:], 1.0)
            nc.gpsimd.affine_select(
                maskT[:], maskT[:],
                pattern=[[-P, 2], [1, IB]],
                compare_op=mybir.AluOpType.is_ge,
                fill=0.0, base=0, channel_multiplier=-1,
            )

            # q/k per d-tile: [128, L] f16 (rows 0:64 head-even, 64:128 head-odd)
            qk_sb = {}
            for nm in ("q", "k"):
                for dt in range(2):
                    qk_sb[(nm, dt)] = pool_qk.tile([P, L], F16, tag=f"{nm}{dt}",
                                                   name=f"{nm}{dt}")
            # v: [128, jt, head, 65] with ones in col 64
            v_sb = pool_v.tile([P, NJT, 4, DK + 1], F16, tag="v")
            nc.gpsimd.memset(v_sb[:, :, :, DK:DK + 1], 1.0)
            # f32 ones row for the PE denominator-broadcast matmul
            ones_r = pool_w.tile([1, DK], F32, tag="ones_r")
            nc.gpsimd.memset(ones_r[:], 1.0)

            # ---- projection emitters (called lazily) ----
            def emit_qk_block(nm, dt, b, nb=1):
                """project q or k for d-tile dt, query blocks [b, b+nb)."""
                w_sb, b_sb = (wq_sb, bq_sb) if nm == "q" else (wk_sb, bk_sb)
                w_ = IB * nb
                pp = pool_pp.tile([P, 2, IB], F32, tag="pp", name="pp")
                ppv = pp[:].rearrange("p a f -> p (a f)")[:, 0:w_]
                for kt in range(NKT):
                    nc.tensor.matmul(
                        ppv,
                        lhsT=w_sb[:, kt, dt * P:(dt + 1) * P],
                        rhs=xts[:, kt, b * IB:b * IB + w_],
                        start=(kt == 0), stop=(kt == NKT - 1),
                    )
                nc.vector.tensor_scalar_add(
                    qk_sb[(nm, dt)][:, b * IB:b * IB + w_], ppv,
                    b_sb[:, dt:dt + 1])

            def emit_v(jt):
                pp = pool_pp.tile([P, 2, IB], F32, tag="pp", name="pp")
                ppv = pp[:, 0, :]
                for kt in range(NKT):
                    nc.tensor.matmul(
                        ppv,
                        lhsT=xts[:, kt, jt * P:(jt + 1) * P],
                        rhs=wv_sb[:, kt, :],
                        start=(kt == 0), stop=(kt == NKT - 1),
                    )
                nc.vector.tensor_copy(
                    v_sb[:, jt, :, 0:DK],
                    ppv.rearrange("p (h e) -> p h e", h=4))

            # ---- attention ----
            zcs = {}

            def attn_pair(b, p, fillers):
                """scores+exp+mask+z for head pair p (heads 2p, 2p+1) of
                block b.  `fillers` is a list of (deadline, fn) lazy emitters
                (projection chunks) interleaved between score groups to keep
                the PE fed while ACT catches up.  Anything due by (b, p) is
                emitted up front -- a PE consumer emitted before its PE
                producer would deadlock the in-order engine stream."""
                due = [f for dl, f in fillers if dl <= (b, p)]
                fillers[:] = [(dl, f) for dl, f in fillers if not dl <= (b, p)]
                for f in due:
                    f()
                nj = 2 * (b + 1)
                qt = qk_sb[("q", p)]
                kt_t = qk_sb[("k", p)]
                # one PSUM bank per head: an open accumulation group must own
                # its bank (start=True clears has_written bank-wide)
                pzc = [pool_pz.tile([P, IB], F32, tag="pz", name=f"pz{c}")
                       for c in range(2)]
                zcs[(b, p)] = (pzc, pool_zc.tile([P, IB], F16, tag="zc",
                                                 name="zc"))
                prev = None

                def emit_z(g, jbs, at):
                    for s, jb in enumerate(jbs):
                        for c in range(2):
                            nc.tensor.matmul(
                                pzc[c][0:DK + 1, :],
                                lhsT=v_sb[:, jb, 2 * p + c, :],
                                rhs=at[:, c, s, :],
                                start=(jb == 0), stop=(jb == nj - 1),
                            )

                ngrp = (nj + 1) // 2
                for g in range(ngrp):
                    jbs = [2 * g + s for s in range(2) if 2 * g + s < nj]
                    ps = pool_ps.tile([P, 2, 2, IB], F32, tag="ps", name="ps")
                    at = pool_at.tile([P, 2, 2, IB], F16, tag="at", name="at")
                    for s, jb in enumerate(jbs):
                        for c in range(2):
                            drow = DK * c
                            nc.tensor.matmul(
                                ps[:, c, s, :],
                                lhsT=kt_t[drow:drow + DK,
                                          jb * P:(jb + 1) * P],
                                rhs=qt[drow:drow + DK,
                                       b * IB:(b + 1) * IB],
                                start=True, stop=True,
                            )
                    if fillers:
                        fillers.pop(0)[1]()
                    if prev is not None:
                        emit_z(*prev)
                    nc.scalar.activation(
                        at[:].rearrange("p a b f -> p (a b f)"),
                        ps[:].rearrange("p a b f -> p (a b f)"),
                        AF.Exp, scale=SCALE)
                    if g == ngrp - 1:
                        # diagonal pair: zero at[j, i] where i < 128*s + j
                        for c in range(2):
                            nc.vector.tensor_mul(at[:, c, :, :],
                                                 at[:, c, :, :], maskT[:])
                    prev = (g, jbs, at)
                emit_z(*prev)

            def norm_front(b, p):
                """z+den -> SBUF (frees pz banks), den row -> partition 0,
                fast reciprocal.  No PE instructions: runs concurrently with
                the next pair's attention."""
                pzc, zc = zcs[(b, p)]
                zsb = pool_nm.tile([P, 2, IB], F32, tag="zsb", name="zsb")
                dent = pool_nm.tile([1, 2, IB], F32, tag="dent", name="dent")
                rden = pool_nm.tile([1, 2, IB], F32, tag="rden", name="rden")
                for c in range(2):
                    nc.vector.tensor_copy(zsb[0:DK + 1, c, :],
                                          pzc[c][0:DK + 1, :])
                nc.gpsimd.dma_start(dent[:], zsb[DK:DK + 1, :, :])
                nc.vector.reciprocal_approx_fast(rden[:], dent[:])
                norms[(b, p)] = (zsb, rden, zc)

            def norm_back(b, p):
                """broadcast 1/den to 64 rows via a tiny f32 PE matmul, then
                multiply.  Deferred through the filler queue so the PE
                instruction never waits on the reciprocal chain."""
                zsb, rden, zc = norms.pop((b, p))
                zn = pool_nm.tile([DK, IB], F16, tag="zn", name="zn")
                bct = pool_pp.tile([P, 2, IB], F32, tag="pp", name="bct")
                nc.tensor.matmul(
                    bct[0:DK, :, :],
                    lhsT=ones_r[:],
                    rhs=rden[:],
                    start=True, stop=True,
                )
                # head-even -> zc rows 0:64; head-odd via SBUF DMA hop
                nc.vector.tensor_mul(zc[0:DK, :], zsb[0:DK, 0, :],
                                     bct[0:DK, 0, :])
                nc.vector.tensor_mul(zn[:], zsb[0:DK, 1, :], bct[0:DK, 1, :])
                nc.gpsimd.dma_start(zc[DK:P, :], zn[:])

            norms = {}

            def outproj_mt(b, mt):
                po = pool_pp.tile([P, IB], F32, tag="pp", name="pp")
                for p in range(2):
                    nc.tensor.matmul(
                        po[:],
                        lhsT=wo_sb[:, p, mt * P:(mt + 1) * P],
                        rhs=zcs[(b, p)][1][:],
                        start=(p == 0), stop=(p == 1),
                    )
                osb = pool_o.tile([P, IB], F32, tag="o", name="o")
                nc.vector.tensor_copy(osb[:], po[:])
                nc.sync.dma_start(
                    outT.ap()[mt * P:(mt + 1) * P, b * IB:(b + 1) * IB],
                    osb[:])

            # ---- emission schedule ----
            # prologue: q/k for blocks 0-1 (512-wide chunks), v for block 0
            for dt in range(2):
                emit_qk_block("q", dt, 0, nb=2)
                emit_qk_block("k", dt, 0, nb=2)
            emit_v(0)
            emit_v(1)

            # global lazy-work queue of (deadline, fn): one item pops per
            # score group so the PE always has dependency-free matmuls while
            # ACT runs exp; anything still pending at its deadline is
            # force-emitted by attn_pair
            END = (NB, 2)
            fillq = []
            for b in range(NB):
                if b + 1 < NB:
                    fillq.append(((b + 1, 0), lambda b=b: emit_v(2 * b + 2)))
                    fillq.append(((b + 1, 0), lambda b=b: emit_v(2 * b + 3)))
                if b % 2 == 0 and b + 2 < NB:
                    # 512-wide q/k chunks for blocks b+2, b+3
                    for dt in range(2):
                        fillq.append(((b + 2, dt),
                                      lambda dt=dt, b=b:
                                      emit_qk_block("q", dt, b + 2, nb=2)))
                        fillq.append(((b + 2, dt),
                                      lambda dt=dt, b=b:
                                      emit_qk_block("k", dt, b + 2, nb=2)))
                if b >= 1:
                    for mt in range(D // P):
                        fillq.append((END,
                                      lambda mt=mt, b=b: outproj_mt(b - 1, mt)))
                attn_pair(b, 0, fillq)
                norm_front(b, 0)
                fillq.append((END, lambda b=b: norm_back(b, 0)))
                attn_pair(b, 1, fillq)
                norm_front(b, 1)
                fillq.append((END, lambda b=b: norm_back(b, 1)))
            for _, f in fillq:
                f()
            for mt in range(D // P):
                outproj_mt(NB - 1, mt)

    nc.compile()
    return nc


_NC = None


def _get_nc():
    global _NC
    if _NC is None:
        _NC = _build()
    return _NC


def _in_maps(x, w_q, b_q, w_k, b_k, w_v, b_v, w_o, b_o):
    maps = []
    for b in range(4):
        xTb = np.ascontiguousarray(x[b].T.astype(np.float16))
        for hg in range(2):
            sl = slice(hg * HD, (hg + 1) * HD)
            maps.append({
                "xT": xTb,
                "wq": np.ascontiguousarray(w_q[sl].T.astype(np.float16)),
                "wk": np.ascontiguousarray(w_k[sl].T.astype(np.float16)),
                "wv": np.ascontiguousarray(w_v[sl].T.astype(np.float16)),
                "wo": np.ascontiguousarray(w_o[:, sl].T.astype(np.float16)),
                "bq": np.ascontiguousarray(b_q[sl].astype(np.float32)),
                "bk": np.ascontiguousarray(b_k[sl].astype(np.float32)),
            })
    return maps


def _combine(results, w_o, b_v, b_o):
    corr = (b_o + w_o @ b_v).astype(np.float32)  # fold v/out biases
    out = np.empty((4, L, D), dtype=np.float32)
    for b in range(4):
        acc = results[2 * b]["outT"] + results[2 * b + 1]["outT"]
        out[b] = acc.T + corr
    return out


def kernel(x, w_q, b_q, w_k, b_k, w_v, b_v, w_o, b_o):
    nc = _get_nc()
    maps = _in_maps(x, w_q, b_q, w_k, b_k, w_v, b_v, w_o, b_o)
    res = run_bass_kernel_spmd(nc, maps, core_ids=list(range(8)))
    return _combine(res.results, w_o, b_v, b_o)


def bench(x, w_q, b_q, w_k, b_k, w_v, b_v, w_o, b_o):
    """Run with NTFF tracing; returns (output, exec_time_ns)."""
    nc = _get_nc()
    maps = _in_maps(x, w_q, b_q, w_k, b_k, w_v, b_v, w_o, b_o)
    res = run_bass_kernel_spmd(nc, maps, core_ids=list(range(8)), trace=True)
    return _combine(res.results, w_o, b_v, b_o), res.exec_time_ns


# revision 32
# speedup vs baseline: 1.2300x; 1.0146x over previous
"""Multi-head causal attention (bs=4, L=2048, d_model=512, 8 heads x 64) on 8
Trainium2 NeuronCores.

Sharding: core c = (batch b = c//2, head-group hg = c%2); each core computes 4
heads of one batch over the full sequence. Host pre-transposes activations and
weight slices so every device matmul has its contraction dim on partitions;
device returns the transposed partial output projection; host sums the two
head-group partials per batch, transposes back and adds the (folded) biases.

v2 layout: 256-wide query blocks (8 of them) for tighter causal coverage.
Per block and head-pair, scores for both heads of a pair land in one 2-bank
PSUM tile [128, 2, 2, 256] so a single ACT exp op covers 2 heads x 2 key
tiles.  The causal mask is a constant 0/1 f16 tile multiplied in on DVE
(diagonal group only).  Softmax denominators ride the z matmul as a ones
column (row 64); normalization: den row -> DMA to partition 0 -> fast
reciprocal -> gpsimd partition_broadcast -> DVE mul straight out of PSUM.
Projection work (QKV, output) is emitted lazily between score groups so the
PE always has dependency-free matmuls to chew on while ACT runs exp.
"""

import numpy as np

import concourse.bacc as bacc
import concourse.mybir as mybir
import concourse.tile as tile
from concourse.bass_utils import run_bass_kernel_spmd

F32 = mybir.dt.float32
F32R = mybir.dt.float32r
F16 = mybir.dt.float16
AF = mybir.ActivationFunctionType

L = 2048          # sequence length
D = 512           # model dim
HD = 256          # head-group output dim (4 heads x 64)
DK = 64           # head dim
P = 128
IB = 256          # query block width
NB = L // IB      # 8 query blocks
NKT = D // P      # 4 contraction tiles over model dim
NJT = L // P      # 16 key tiles
SCALE = 1.0 / 8.0  # 1/sqrt(DK)


def _build():
    nc = bacc.Bacc("TRN2", target_bir_lowering=False, debug=False,
                   enable_asserts=False)

    xT = nc.dram_tensor("xT", [D, L], F16, kind="ExternalInput")
    wq = nc.dram_tensor("wq", [D, HD], F16, kind="ExternalInput")
    wk = nc.dram_tensor("wk", [D, HD], F16, kind="ExternalInput")
    wv = nc.dram_tensor("wv", [D, HD], F16, kind="ExternalInput")
    wo = nc.dram_tensor("wo", [HD, D], F16, kind="ExternalInput")
    bq = nc.dram_tensor("bq", [HD], F32, kind="ExternalInput")
    bk = nc.dram_tensor("bk", [HD], F32, kind="ExternalInput")
    outT = nc.dram_tensor("outT", [D, L], F32, kind="ExternalOutput")

    with tile.TileContext(nc) as tc:
        with (
            tc.tile_pool(name="w", bufs=1) as pool_w,
            tc.tile_pool(name="x", bufs=1) as pool_x,
            tc.tile_pool(name="qk", bufs=1) as pool_qk,
            tc.tile_pool(name="v", bufs=1) as pool_v,
            tc.tile_pool(name="at", bufs=3) as pool_at,
            tc.tile_pool(name="nm", bufs=3) as pool_nm,
            tc.tile_pool(name="zc", bufs=4) as pool_zc,
            tc.tile_pool(name="o", bufs=2) as pool_o,
            tc.tile_pool(name="ps", bufs=1, space="PSUM") as pool_ps,
            tc.tile_pool(name="pz", bufs=4, space="PSUM") as pool_pz,
            tc.tile_pool(name="pp", bufs=2, space="PSUM") as pool_pp,
        ):
            # ---- static tiles ----
            wq_sb = pool_w.tile([P, NKT, HD], F16, tag="wq")
            wk_sb = pool_w.tile([P, NKT, HD], F16, tag="wk")
            wv_sb = pool_w.tile([P, NKT, HD], F16, tag="wv")
            wo_sb = pool_w.tile([P, HD // P, D], F16, tag="wo")
            bq_sb = pool_w.tile([P, HD // P], F32, tag="bq")
            bk_sb = pool_w.tile([P, HD // P], F32, tag="bk")
            # interleave weight and x-chunk arrivals so the first projection
            # matmuls can start as soon as wq + one x column-chunk land
            xts = pool_x.tile([P, NKT, L], F16, tag="x")

            def dma_x(b):
                for kt in range(NKT):
                    nc.sync.dma_start(
                        xts[:, kt, b * IB:(b + 1) * IB],
                        xT.ap()[kt * P:(kt + 1) * P, b * IB:(b + 1) * IB])

            nc.sync.dma_start(wq_sb[:], wq.ap().rearrange("(t p) n -> p t n", p=P))
            nc.sync.dma_start(bq_sb[:], bq.ap().rearrange("(t p) -> p t", p=P))
            dma_x(0)
            dma_x(1)
            nc.sync.dma_start(wk_sb[:], wk.ap().rearrange("(t p) n -> p t n", p=P))
            nc.sync.dma_start(bk_sb[:], bk.ap().rearrange("(t p) -> p t", p=P))
            dma_x(2)
            nc.sync.dma_start(wv_sb[:], wv.ap().rearrange("(t p) n -> p t n", p=P))
            dma_x(3)
            nc.sync.dma_start(wo_sb[:], wo.ap().rearrange("(t p) n -> p t n", p=P))
            for b in range(4, NB):
                dma_x(b)

            # causal mask for the diagonal k-tile pair: keep iff i >= 128*t + p
            maskT = pool_w.tile([P, 2, IB], F16, tag="maskT")
            nc.gpsimd.memset(maskT[:], 1.0)
            nc.gpsimd.affine_select(
                maskT[:], maskT[:],
                pattern=[[-P, 2], [1, IB]],
                compare_op=mybir.AluOpType.is_ge,
                fill=0.0, base=0, channel_multiplier=-1,
            )

            # q/k per d-tile: [128, L] f16 (rows 0:64 head-even, 64:128 head-odd)
            qk_sb = {}
            for nm in ("q", "k"):
                for dt in range(2):
                    qk_sb[(nm, dt)] = pool_qk.tile([P, L], F16, tag=f"{nm}{dt}",
                                                   name=f"{nm}{dt}")
            # v: [128, jt, head, 65] with ones in col 64
            v_sb = pool_v.tile([P, NJT, 4, DK + 1], F16, tag="v")
            nc.gpsimd.memset(v_sb[:, :, :, DK:DK + 1], 1.0)
            # f32 ones row for the PE denominator-broadcast matmul
            ones_r = pool_w.tile([1, DK], F32, tag="ones_r")
            nc.gpsimd.memset(ones_r[:], 1.0)

            # ---- projection emitters (called lazily) ----
            def emit_qk_block(nm, dt, b, nb=1):
                """project q or k for d-tile dt, query blocks [b, b+nb)."""
                w_sb, b_sb = (wq_sb, bq_sb) if nm == "q" else (wk_sb, bk_sb)
                w_ = IB * nb
                pp = pool_pp.tile([P, 2, IB], F32, tag="pp", name="pp")
                ppv = pp[:].rearrange("p a f -> p (a f)")[:, 0:w_]
                for kt in range(NKT):
                    nc.tensor.matmul(
                        ppv,
                        lhsT=w_sb[:, kt, dt * P:(dt + 1) * P],
                        rhs=xts[:, kt, b * IB:b * IB + w_],
                        start=(kt == 0), stop=(kt == NKT - 1),
                    )
                nc.vector.tensor_scalar_add(
                    qk_sb[(nm, dt)][:, b * IB:b * IB + w_], ppv,
                    b_sb[:, dt:dt + 1])

            def emit_v(jt):
                pp = pool_pp.tile([P, 2, IB], F32, tag="pp", name="pp")
                ppv = pp[:, 0, :]
                for kt in range(NKT):
                    nc.tensor.matmul(
                        ppv,
                        lhsT=xts[:, kt, jt * P:(jt + 1) * P],
                        rhs=wv_sb[:, kt, :],
                        start=(kt == 0), stop=(kt == NKT - 1),
                    )
                nc.vector.tensor_copy(
                    v_sb[:, jt, :, 0:DK],
                    ppv.rearrange("p (h e) -> p h e", h=4))

            # ---- attention ----
            zcs = {}

            def attn_pair(b, p, fillers):
                """scores+exp+mask+z for head pair p (heads 2p, 2p+1) of
                block b.  `fillers` is a list of (deadline, fn) lazy emitters
                (projection chunks) interleaved between score groups to keep
                the PE fed while ACT catches up.  Anything due by (b, p) is
                emitted up front -- a PE consumer emitted before its PE
                producer would deadlock the in-order engine stream."""
                due = [f for dl, f in fillers if dl <= (b, p)]
                fillers[:] = [(dl, f) for dl, f in fillers if not dl <= (b, p)]
                for f in due:
                    f()
                nj = 2 * (b + 1)
                qt = qk_sb[("q", p)]
                kt_t = qk_sb[("k", p)]
                # one PSUM bank per head: an open accumulation group must own
                # its bank (start=True clears has_written bank-wide)
                pzc = [pool_pz.tile([P, IB], F32, tag="pz", name=f"pz{c}")
                       for c in range(2)]
                zcs[(b, p)] = (pzc, pool_zc.tile([P, IB], F16, tag="zc",
                                                 name="zc"))
                prev = None

                def emit_z(g, jbs, at):
                    for s, jb in enumerate(jbs):
                        for c in range(2):
                            nc.tensor.matmul(
                                pzc[c][0:DK + 1, :],
                                lhsT=v_sb[:, jb, 2 * p + c, :],
                                rhs=at[:, c, s, :],
                                start=(jb == 0), stop=(jb == nj - 1),
                            )

                ngrp = (nj + 1) // 2
                for g in range(ngrp):
                    jbs = [2 * g + s for s in range(2) if 2 * g + s < nj]
                    ps = pool_ps.tile([P, 2, 2, IB], F32, tag="ps", name="ps")
                    at = pool_at.tile([P, 2, 2, IB], F16, tag="at", name="at")
                    for s, jb in enumerate(jbs):
                        for c in range(2):
                            drow = DK * c
                            nc.tensor.matmul(
                                ps[:, c, s, :],
                                lhsT=kt_t[drow:drow + DK,
                                          jb * P:(jb + 1) * P],
                                rhs=qt[drow:drow + DK,
                                       b * IB:(b + 1) * IB],
                                start=True, stop=True,
                            )
                    if fillers:
                        fillers.pop(0)[1]()
                    if prev is not None:
                        emit_z(*prev)
                    nc.scalar.activation(
                        at[:].rearrange("p a b f -> p (a b f)"),
                        ps[:].rearrange("p a b f -> p (a b f)"),
                        AF.Exp, scale=SCALE)
                    if g == ngrp - 1:
                        # diagonal pair: zero at[j, i] where i < 128*s + j
                        for c in range(2):
                            nc.vector.tensor_mul(at[:, c, :, :],
                                                 at[:, c, :, :], maskT[:])
                    prev = (g, jbs, at)
                emit_z(*prev)

            def norm_front(b, p):
                """den row -> SBUF -> partition 0, fast reciprocal, then
                gpsimd broadcast of 1/den to 64 partitions.  No PE
                instructions: runs concurrently with the next pair."""
                pzc, zc = zcs[(b, p)]
                dsb = pool_nm.tile([P, 2, IB], F32, tag="dsb", name="dsb")
                dent = pool_nm.tile([1, 2, IB], F32, tag="dent", name="dent")
                rden = pool_nm.tile([1, 2, IB], F32, tag="rden", name="rden")
                bct = pool_nm.tile([DK, 2, IB], F32, tag="bct", name="bct")
                for c in range(2):
                    nc.vector.tensor_copy(dsb[DK:DK + 1, c, :],
                                          pzc[c][DK:DK + 1, :])
                nc.sync.dma_start(dent[:], dsb[DK:DK + 1, :, :])
                nc.vector.reciprocal_approx_fast(rden[:], dent[:])
                nc.gpsimd.partition_broadcast(bct[:], rden[:], channels=DK)
                norms[(b, p)] = (bct, zc)

            def norm_back(b, p):
                """z * (1/den) straight out of PSUM.  Deferred through the
                filler queue so the in-order DVE stream never waits on the
                reciprocal/broadcast chain."""
                bct, zc = norms.pop((b, p))
                pzc, _ = zcs[(b, p)]
                zn = pool_nm.tile([DK, IB], F16, tag="zn", name="zn")
                # head-even -> zc rows 0:64; head-odd via SBUF DMA hop
                nc.vector.tensor_mul(zc[0:DK, :], bct[:, 0, :],
                                     pzc[0][0:DK, :])
                nc.vector.tensor_mul(zn[:], bct[:, 1, :], pzc[1][0:DK, :])
                nc.sync.dma_start(zc[DK:P, :], zn[:])

            norms = {}

            def outproj_mt(b, mt):
                po = pool_pp.tile([P, IB], F32, tag="pp", name="pp")
                for p in range(2):
                    nc.tensor.matmul(
                        po[:],
                        lhsT=wo_sb[:, p, mt * P:(mt + 1) * P],
                        rhs=zcs[(b, p)][1][:],
                        start=(p == 0), stop=(p == 1),
                    )
                osb = pool_o.tile([P, IB], F32, tag="o", name="o")
                nc.vector.tensor_copy(osb[:], po[:])
                nc.sync.dma_start(
                    outT.ap()[mt * P:(mt + 1) * P, b * IB:(b + 1) * IB],
                    osb[:])

            # ---- emission schedule ----
            # prologue: q/k for blocks 0-1 (512-wide chunks), v for block 0
            for dt in range(2):
                emit_qk_block("q", dt, 0, nb=2)
                emit_qk_block("k", dt, 0, nb=2)
            emit_v(0)
            emit_v(1)

            # global lazy-work queue of (deadline, fn): one item pops per
            # score group so the PE always has dependency-free matmuls while
            # ACT runs exp; anything still pending at its deadline is
            # force-emitted by attn_pair
            END = (NB, 2)
            fillq = []
            for b in range(NB):
                if b + 1 < NB:
                    fillq.append(((b + 1, 0), lambda b=b: emit_v(2 * b + 2)))
                    fillq.append(((b + 1, 0), lambda b=b: emit_v(2 * b + 3)))
                if b % 2 == 0 and b + 2 < NB:
                    # 512-wide q/k chunks for blocks b+2, b+3
                    for dt in range(2):
                        fillq.append(((b + 2, dt),
                                      lambda dt=dt, b=b:
                                      emit_qk_block("q", dt, b + 2, nb=2)))
                        fillq.append(((b + 2, dt),
                                      lambda dt=dt, b=b:
                                      emit_qk_block("k", dt, b + 2, nb=2)))
                if b >= 1:
                    for mt in range(D // P):
                        fillq.append((END,
                                      lambda mt=mt, b=b: outproj_mt(b - 1, mt)))
                attn_pair(b, 0, fillq)
                norm_front(b, 0)
                fillq.append((END, lambda b=b: norm_back(b, 0)))
                attn_pair(b, 1, fillq)
                norm_front(b, 1)
                fillq.append((END, lambda b=b: norm_back(b, 1)))
            for _, f in fillq:
                f()
            for mt in range(D // P):
                outproj_mt(NB - 1, mt)

    nc.compile()
    return nc


_NC = None


def _get_nc():
    global _NC
    if _NC is None:
        _NC = _build()
    return _NC


def _in_maps(x, w_q, b_q, w_k, b_k, w_v, b_v, w_o, b_o):
    maps = []
    for b in range(4):
        xTb = np.ascontiguousarray(x[b].T.astype(np.float16))
        for hg in range(2):
            sl = slice(hg * HD, (hg + 1) * HD)
            maps.append({
                "xT": xTb,
                "wq": np.ascontiguousarray(w_q[sl].T.astype(np.float16)),
                "wk": np.ascontiguousarray(w_k[sl].T.astype(np.float16)),
                "wv": np.ascontiguousarray(w_v[sl].T.astype(np.float16)),
                "wo": np.ascontiguousarray(w_o[:, sl].T.astype(np.float16)),
                "bq": np.ascontiguousarray(b_q[sl].astype(np.float32)),
                "bk": np.ascontiguousarray(b_k[sl].astype(np.float32)),
            })
    return maps


def _combine(results, w_o, b_v, b_o):
    corr = (b_o + w_o @ b_v).astype(np.float32)  # fold v/out biases
    out = np.empty((4, L, D), dtype=np.float32)
    for b in range(4):
        acc = results[2 * b]["outT"] + results[2 * b + 1]["outT"]
        out[b] = acc.T + corr
    return out


def kernel(x, w_q, b_q, w_k, b_k, w_v, b_v, w_o, b_o):
    nc = _get_nc()
    maps = _in_maps(x, w_q, b_q, w_k, b_k, w_v, b_v, w_o, b_o)
    res = run_bass_kernel_spmd(nc, maps, core_ids=list(range(8)))
    return _combine(res.results, w_o, b_v, b_o)


def bench(x, w_q, b_q, w_k, b_k, w_v, b_v, w_o, b_o):
    """Run with NTFF tracing; returns (output, exec_time_ns)."""
    nc = _get_nc()
    maps = _in_maps(x, w_q, b_q, w_k, b_k, w_v, b_v, w_o, b_o)
    res = run_bass_kernel_spmd(nc, maps, core_ids=list(range(8)), trace=True)
    return _combine(res.results, w_o, b_v, b_o), res.exec_time_ns
